# revision 1
# baseline (speedup 1.0000x reference)
"""Trainium2 Bass kernel for AdvancedNeuralMemory (B=4, S=8192, D=1024, M=512).

Math notes
----------
The recurrence  s_t = g * s_{t-1} + u_t  has a *scalar constant* gate
g = sigmoid(forget_factor) ~= 0.525, so  mem_t = sum_{j<=t} g^(t-j) u_j.
g^129 ~ 7e-37, far below fp32 resolution, so a 256-step window is exact in
fp32: for 128-row time tiles,
    mem_tile_i = Tprev.T @ u_{i-1} + Tcur.T @ u_i
with host-precomputed decay-Toeplitz matrices (adaptive_lr folded in).
This removes the sequential dependency entirely -> pure matmuls.

Sharding: 8 cores = (batch 0..3) x (seq half 0..1). Each core processes a
[4096, 1024] slab plus a 128-row halo tile (for u_{i-1} of its first tile).
No cross-device communication.

Layout: sequence-on-partitions ([128 s-rows, features] tiles); matmul
contractions get their lhsT via PE transposes. All big matmuls run as
float32r (full fp32 data, 1 cycle/row when free dim >= 256).
"""

import sys
import os

for _p in ("/opt/trn_rl_repo",):
    if _p not in sys.path and os.path.isdir(_p):
        sys.path.insert(0, _p)

from contextlib import ExitStack

import numpy as np

import concourse.bass as bass
import concourse.mybir as mybir
import concourse.tile as tile
from concourse.bass_utils import run_bass_kernel_spmd

B, S, D, M = 4, 8192, 1024, 512
HALF = S // 2          # rows per core
TS = 128               # s-tile rows
NT = HALF // TS        # compute tiles per core (32)
SLAB = HALF + TS       # slab rows incl. halo tile
LN_EPS = 1e-5
N_CORES = 8
# packed weights: wd(8*512) wq/wk/wv/w1/w2(4*512 each) wu(4*1024) tt(2*128)
WPACK_COLS = 8 * M + 5 * 4 * M + 4 * D + 2 * TS

f32 = mybir.dt.float32
f32r = mybir.dt.float32r
AF = mybir.ActivationFunctionType
ALU = mybir.AluOpType

# test.py can flip these
TRACE = False
TRACE_KWARGS = {}
LAST_RESULTS = None    # BassKernelResults of the last run (exec_time_ns etc.)

_PROG_CACHE = {}


def _r(ap):
    """View an fp32 AP as float32r for full-rate PE matmuls."""
    return ap.bitcast(f32r)


def _fix_matmult_waits(nc):
    """Walrus allows only one sync-wait on a (fused-ldweights) Matmult.
    Move surplus waits onto an inserted NoOp on the same engine."""
    n = 0
    for f in nc.m.functions:
        for bb in f.blocks:
            insts = bb.instructions
            i = 0
            while i < len(insts):
                inst = insts[i]
                si = inst.sync_info
                tname = type(inst).__name__
                exempt = tname in ("InstNoOp",
                                   "InstEventSemaphore",
                                   "InstUnconditionalBranch", "InstCall",
                                   "InstISA", "InstRegisterMove")
                if (not exempt and si is not None and si.on_wait
                        and len(si.on_wait) > 1):
                    for w in list(si.on_wait[:-1]):
                        nop = mybir.InstNoOp(
                            name=f"wfix-{n}", ins=[], outs=[],
                            engine=inst.engine,
                            sync_info=mybir.SyncInfo(on_wait=[w],
                                                     on_update=[]))
                        insts.insert(i, nop)
                        n += 1
                        i += 1
                    si.on_wait = [si.on_wait[-1]]
                i += 1
    return n


def _build_program(flags):
    (has_bd, has_bq, has_bk, has_bv, has_gq, has_bqln, has_gk, has_bkln,
     has_bu) = flags
    nc = bass.Bass()

    x_slab = nc.declare_dram_parameter("x_slab", [SLAB, D], f32, isOutput=False)
    wpack = nc.declare_dram_parameter("wpack", [TS, WPACK_COLS], f32r,
                                      isOutput=False)
    ident = nc.declare_dram_parameter("ident", [TS, TS], f32, isOutput=False)
    hmask = nc.declare_dram_parameter("hmask", [TS, 1], f32, isOutput=False)
    opt = {}
    for name, used, shape in (
        ("bd_b", has_bd, [TS, M]), ("bq_b", has_bq, [TS, M]),
        ("bk_b", has_bk, [TS, M]), ("bv_b", has_bv, [TS, M]),
        ("gq_b", has_gq, [TS, M]), ("bqln_b", has_bqln, [TS, M]),
        ("gk_b", has_gk, [TS, M]), ("bkln_b", has_bkln, [TS, M]),
        ("bu_b", has_bu, [TS, D]),
    ):
        if used:
            opt[name] = nc.declare_dram_parameter(name, shape, f32,
                                                  isOutput=False)
    y = nc.declare_dram_parameter("y", [HALF, D], f32, isOutput=True)

    with tile.TileContext(nc) as tc, ExitStack() as ctx:
        wpool = ctx.enter_context(tc.tile_pool(name="weights", bufs=1))

        wp_sb = wpool.tile([TS, WPACK_COLS], f32r)
        nc.sync.dma_start(wp_sb[:], wpack[:])
        _off = [0]

        def _wseg(nk, ncols):
            a = _off[0]
            _off[0] += nk * ncols
            return wp_sb[:, a:_off[0]].rearrange("p (k m) -> p k m", k=nk)

        wd_sb = _wseg(8, M)
        wq_sb = _wseg(4, M)
        wk_sb = _wseg(4, M)
        wv_sb = _wseg(4, M)
        w1_sb = _wseg(4, M)
        w2_sb = _wseg(4, M)
        wu_sb = _wseg(4, D)
        tt_sb = _wseg(2, TS)
        id_sb = wpool.tile([TS, TS], f32)
        nc.sync.dma_start(id_sb[:], ident[:])
        hm_sb = wpool.tile([TS, 1], f32)
        nc.sync.dma_start(hm_sb[:], hmask[:])
        eps_sb = wpool.tile([TS, 1], f32)
        nc.vector.memset(eps_sb[:], LN_EPS)
        opt_sb = {}
        for name, h in opt.items():
            t = wpool.tile([TS, h.shape[1]], f32, tag=name, name=name)
            nc.sync.dma_start(t[:], h[:])
            opt_sb[name] = t

        # SBUF activation pools
        p_x = ctx.enter_context(tc.tile_pool(name="x", bufs=3))
        p_y = ctx.enter_context(tc.tile_pool(name="y", bufs=3))
        p_xT = ctx.enter_context(tc.tile_pool(name="xT", bufs=2))
        p_tT = ctx.enter_context(tc.tile_pool(name="tT", bufs=2))
        p_act = ctx.enter_context(tc.tile_pool(name="act", bufs=3))
        p_u = ctx.enter_context(tc.tile_pool(name="u", bufs=3))
        p_sm = ctx.enter_context(tc.tile_pool(name="sm", bufs=6))
        # PSUM pools (8 banks total: 3 + 2 + 2 = 7 used)
        p_mm = ctx.enter_context(tc.tile_pool(name="mm", bufs=4, space="PSUM"))
        p_pt = ctx.enter_context(tc.tile_pool(name="pt", bufs=2, space="PSUM"))
        p_out = ctx.enter_context(
            tc.tile_pool(name="out", bufs=1, space="PSUM"))

        def pe_transpose(src_sb, nblk, tag):
            """Transpose nblk [128,128] blocks of src_sb into a fresh SBUF
            tile laid out [128, nblk*128] (feature-on-partition)."""
            pool = p_tT if nblk <= 4 else p_xT
            dst = pool.tile([TS, nblk * TS], f32r, tag=tag, name=tag)
            for g0 in range(0, nblk, 4):
                gn = min(4, nblk - g0)
                ps = p_pt.tile([TS, 4 * TS], f32, name='ps_t', tag='ps_t')
                for j in range(gn):
                    blk = slice((g0 + j) * TS, (g0 + j + 1) * TS)
                    nc.tensor.transpose(ps[:, j * TS:(j + 1) * TS],
                                        src_sb[:, blk], id_sb[:])
                for j in range(gn):
                    dst_sl = dst[:, (g0 + j) * TS:(g0 + j + 1) * TS]
                    src_sl = ps[:, j * TS:(j + 1) * TS]
                    nc.scalar.copy(dst_sl, src_sl)
            return dst

        def layernorm(z_ps, gb, bb, tag):
            """LN over free dim of z_ps [128, M] (PSUM) -> SBUF tile."""
            st = p_sm.tile([TS, 6], f32, tag="bnst")
            nc.vector.bn_stats(st[:], z_ps[:])
            ag = p_sm.tile([TS, 2], f32, tag="bnag")
            nc.vector.bn_aggr(ag[:], st[:])
            std = p_sm.tile([TS, 1], f32, tag="std")
            nc.scalar.activation(std[:], ag[:, 1:2], AF.Sqrt, bias=eps_sb[:])
            rs = p_sm.tile([TS, 1], f32, tag="rs")
            nc.vector.reciprocal(rs[:], std[:])
            nmr = p_sm.tile([TS, 1], f32, tag="nmr")
            nc.vector.scalar_tensor_tensor(nmr[:], ag[:, 0:1], -1.0, rs[:],
                                           ALU.mult, ALU.mult)
            o = p_act.tile([TS, M], f32, tag=tag, name=tag)
            nc.scalar.activation(o[:], z_ps[:], AF.Identity,
                                 bias=nmr[:], scale=rs[:])
            if gb is not None:
                nc.vector.tensor_mul(o[:], o[:], gb[:])
            if bb is not None:
                nc.vector.tensor_add(o[:], o[:], bb[:])
            return o

        def mm_acc(out_ps, lhsT_sb, rhs_sb_3d, nk, ncols=M):
            for k in range(nk):
                nc.tensor.matmul(
                    out_ps[:, 0:ncols],
                    lhsT_sb[:, k * TS:(k + 1) * TS],
                    rhs_sb_3d[:, k, 0:ncols],
                    start=(k == 0), stop=(k == nk - 1))

        u_prev = None
        for i in range(NT + 1):
            halo = (i == 0)
            s0 = i * TS

            xt = p_x.tile([TS, D], f32)
            nc.sync.dma_start(xt[:], x_slab[s0:s0 + TS, :])

            xT = pe_transpose(xt, 8, "xT")

            h_ps = p_mm.tile([TS, M], f32, tag="mm", name="h_ps")
            mm_acc(h_ps, xT, wd_sb, 8)
            h_sb = p_act.tile([TS, M], f32, tag="h")
            nc.scalar.copy(h_sb[:], h_ps[:])
            if has_bd:
                nc.vector.tensor_add(h_sb[:], h_sb[:], opt_sb["bd_b"][:])
            hT = pe_transpose(h_sb, 4, "hT")

            if not halo:
                zq_ps = p_mm.tile([TS, M], f32, tag="mm", name="zq_ps")
                mm_acc(zq_ps, hT, wq_sb, 4)
                if has_bq:
                    nc.vector.tensor_add(zq_ps[:], zq_ps[:],
                                         opt_sb["bq_b"][:])
                q_sb = layernorm(zq_ps,
                                 opt_sb.get("gq_b"), opt_sb.get("bqln_b"),
                                 "q")

            zk_ps = p_mm.tile([TS, M], f32, tag="mm", name="zk_ps")
            mm_acc(zk_ps, hT, wk_sb, 4)
            if has_bk:
                nc.vector.tensor_add(zk_ps[:], zk_ps[:], opt_sb["bk_b"][:])
            k_sb = layernorm(zk_ps, opt_sb.get("gk_b"), opt_sb.get("bkln_b"),
                             "k")

            zv_ps = p_mm.tile([TS, M], f32, tag="mm", name="zv_ps")
            mm_acc(zv_ps, hT, wv_sb, 4)
            v_sb = p_act.tile([TS, M], f32, tag="v")
            nc.scalar.copy(v_sb[:], zv_ps[:])
            if has_bv:
                nc.vector.tensor_add(v_sb[:], v_sb[:], opt_sb["bv_b"][:])

            kT = pe_transpose(k_sb, 4, "kT")
            a1_ps = p_mm.tile([TS, M], f32, tag="mm", name="a1_ps")
            mm_acc(a1_ps, kT, w1_sb, 4)
            a1_sb = p_act.tile([TS, M], f32, tag="a1")
            nc.scalar.activation(a1_sb[:], a1_ps[:], AF.Gelu_apprx_tanh)
            a1T = pe_transpose(a1_sb, 4, "a1T")
            pred_ps = p_mm.tile([TS, M], f32, tag="mm", name="pred_ps")
            mm_acc(pred_ps, a1T, w2_sb, 4)

            u_sb = p_u.tile([TS, M], f32r)
            nc.vector.tensor_sub(u_sb[:], v_sb[:], pred_ps[:])
            if halo:
                nc.vector.tensor_scalar_mul(u_sb[:], u_sb[:], hm_sb[:, 0:1])
                u_prev = u_sb
                continue

            mem_ps = p_mm.tile([TS, M], f32, tag="mm", name="mem_ps")
            nc.tensor.matmul(mem_ps[:], tt_sb[:, 0, :], u_prev[:],
                             start=True, stop=False)
            nc.tensor.matmul(mem_ps[:], tt_sb[:, 1, :], u_sb[:],
                             start=False, stop=True)
            u_prev = u_sb

            rtr_sb = p_act.tile([TS, M], f32, tag="rtr")
            nc.vector.tensor_mul(rtr_sb[:], q_sb[:], mem_ps[:])
            rT = pe_transpose(rtr_sb, 4, "rT")

            out_ps = p_out.tile([TS, D], f32)
            for nb in range(2):
                cols = slice(nb * 512, (nb + 1) * 512)
                for k in range(4):
                    nc.tensor.matmul(
                        out_ps[:, cols],
                        rT[:, k * TS:(k + 1) * TS],
                        wu_sb[:, k, cols],
                        start=(k == 0), stop=(k == 3))

            y_sb = p_y.tile([TS, D], f32)
            nc.vector.tensor_add(y_sb[:], xt[:], out_ps[:])
            if has_bu:
                nc.vector.tensor_add(y_sb[:], y_sb[:], opt_sb["bu_b"][:])
            nc.sync.dma_start(y[s0 - TS:s0, :], y_sb[:])

    _fix_matmult_waits(nc)
    return nc


def _prep_inputs(x, Wd, bd, Wq, bq, Wk, bk, Wv, bv, gq, bq_ln, gk, bk_ln,
                 W1, W2, Wu, bu, adaptive_lr, forget_factor):
    """Host-side: flags, decay matrix, per-core slabs."""
    f = np.float32
    bd, bq, bk, bv, bu = (np.asarray(a, f) for a in (bd, bq, bk, bv, bu))
    gq, bq_ln, gk, bk_ln = (np.asarray(a, f) for a in (gq, bq_ln, gk, bk_ln))
    flags = (bool(bd.any()), bool(bq.any()), bool(bk.any()), bool(bv.any()),
             bool((gq != 1).any()), bool(bq_ln.any()),
             bool((gk != 1).any()), bool(bk_ln.any()), bool(bu.any()))

    g = 1.0 / (1.0 + np.exp(-np.float64(forget_factor)))
    lr = np.float64(adaptive_lr)
    t_idx = np.arange(TS)
    # current-tile block: coeff for u_cur[j] at output t: g^(t-j), j <= t
    lag_cur = t_idx[:, None] - t_idx[None, :]
    Tcur = np.where(lag_cur >= 0, g ** np.maximum(lag_cur, 0), 0.0) * lr
    # previous-tile block: coeff for u_prev[j]: g^(t+128-j)
    lag_prev = t_idx[:, None] + TS - t_idx[None, :]
    Tprev = (g ** lag_prev) * lr
    TT = np.concatenate([Tprev, Tcur], axis=1).T.astype(f)  # [256, 128]
    TT = np.ascontiguousarray(TT)

    def seg(w):
        w = np.asarray(w, f)          # [K, N] -> [128, nk*N]
        nk = w.shape[0] // TS
        return w.reshape(nk, TS, w.shape[1]).transpose(1, 0, 2).reshape(TS, -1)

    wpack = np.ascontiguousarray(np.concatenate(
        [seg(w) for w in (Wd, Wq, Wk, Wv, W1, W2, Wu, TT)], axis=1))
    common = {
        "wpack": wpack,
        "ident": np.eye(TS, dtype=f),
    }
    names = ("bd_b", "bq_b", "bk_b", "bv_b", "gq_b", "bqln_b", "gk_b",
             "bkln_b", "bu_b")
    vecs = (bd, bq, bk, bv, gq, bq_ln, gk, bk_ln, bu)
    for name, used, vec in zip(names, flags, vecs):
        if used:
            common[name] = np.ascontiguousarray(
                np.broadcast_to(vec, (TS, vec.shape[0])), f)

    x = np.asarray(x, f)
    in_maps = []
    for c in range(N_CORES):
        b, sh = c // 2, c % 2
        if sh == 0:
            haloblk = np.zeros((TS, D), f)
            hm = np.zeros((TS, 1), f)
        else:
            haloblk = x[b, HALF - TS:HALF]
            hm = np.ones((TS, 1), f)
        slab = np.concatenate([haloblk, x[b, sh * HALF:(sh + 1) * HALF]],
                              axis=0)
        m = dict(common)
        m["x_slab"] = np.ascontiguousarray(slab)
        m["hmask"] = hm
        in_maps.append(m)
    return flags, in_maps


def kernel(**inputs):
    global LAST_RESULTS
    flags, in_maps = _prep_inputs(**inputs)
    if flags not in _PROG_CACHE:
        _PROG_CACHE[flags] = _build_program(flags)
    nc = _PROG_CACHE[flags]

    res = run_bass_kernel_spmd(nc, in_maps, list(range(N_CORES)),
                               trace=TRACE, trace_kwargs=TRACE_KWARGS)
    LAST_RESULTS = res

    x = np.asarray(inputs["x"], np.float32)
    out = np.empty((B, S, D), np.float32)
    for c in range(N_CORES):
        b, sh = c // 2, c % 2
        out[b, sh * HALF:(sh + 1) * HALF] = res.results[c]["y"]
    return out


if __name__ == "__main__":
    rng = np.random.default_rng(0)
    print("smoke test with random inputs (not the reference distribution)")



# revision 67
# speedup vs baseline: 3.5537x; 3.5537x over previous
"""Trainium2 Bass kernel for AdvancedNeuralMemory (B=4, S=8192, D=1024, M=512).

Math
----
s_t = g*s_{t-1} + u_t with scalar g = sigmoid(forget_factor) ~ 0.525.
g^129 < fp32 eps, so mem for a 128-row tile is exactly
    mem_i = Tprev.T @ u_{i-1} + Tcur.T @ u_i
with host-precomputed decay-Toeplitz matrices (adaptive_lr folded in).
Sequential scan -> pure matmuls; 8 cores = (batch 0..3) x (seq half 0..1),
each works a [4096,1024] slab + one 128-row halo tile. No cross-core comm.

V2 design (from trace analysis of the fp32 baseline @ 978us):
 * bf16 operands everywhere on the PE (fp32 PSUM accumulate). Inputs are
   cast host-side; residual add uses an fp32 copy of x; y stays fp32.
 * hT and a1T computed directly in transposed orientation with the
   *weights* as the stationary operand over 512-row macro-tiles:
   kills the h- and a1- PE transposes and their PSUM->SBUF copies.
 * LN inv-std via DVE Newton rsqrt (bit-hack seed): the scalar engine
   never touches the Sqrt table set, so the Gelu table stays resident
   (the baseline lost ~5.4us/tile to ACT_TABLE_LOAD thrash).
 * macro-level software pipeline: A(g) = x/xT/hT/qkv/LN/kT,
   B(g) = a1T/pred/u/mem/rtr/rT/out/y, issued A0 A1 B0 A2 B1 ... so the
   PE never waits on the LN->kT chain of the current group.
"""

import sys
import os

for _p in ("/opt/trn_rl_repo",):
    if _p not in sys.path and os.path.isdir(_p):
        sys.path.insert(0, _p)

from contextlib import ExitStack

import numpy as np
import ml_dtypes

import concourse.bass as bass
import concourse.mybir as mybir
import concourse.tile as tile
from concourse.bass_utils import run_bass_kernel_spmd

B, S, D, M = 4, 8192, 1024, 512
HALF = S // 2          # rows per core
TS = 128               # s-tile rows
NT = HALF // TS        # compute tiles per core (32)
SLAB = HALF + TS       # slab rows incl. halo tile
LN_EPS = 1e-5
N_CORES = 8
GW = 8                 # tiles per macro-group
# groups of tile indices: [0..3],[4..7],...,[28..31],[32]
GROUPS = [(g, min(GW, NT + 1 - g)) for g in range(0, NT + 1, GW)]

f32 = mybir.dt.float32
bf = mybir.dt.bfloat16
fp8 = mybir.dt.float8e4
u32 = mybir.dt.uint32
AF = mybir.ActivationFunctionType
ALU = mybir.AluOpType
DR = mybir.MatmulPerfMode.DoubleRow
np_bf16 = ml_dtypes.bfloat16
np_fp8 = ml_dtypes.float8_e4m3
SCL = 64.0            # mem scaling so fp8 operands sit in normal range

# packed bf16 weights: wd(8*512) wq/wk/wv/w1/w2(4*512) wu(4*1024) tt(2*128)
WPACK_COLS = 8 * M + 5 * 4 * M + 4 * D + 2 * TS

TRACE = False
TRACE_KWARGS = {}
LAST_RESULTS = None

_PROG_CACHE = {}

MAGIC = 0x5F3759DF


def _fix_matmult_waits(nc):
    """Walrus allows only one sync-wait on a (fused-ldweights) Matmult.
    Move surplus waits onto an inserted NoOp on the same engine."""
    n = 0
    for f in nc.m.functions:
        for bb in f.blocks:
            insts = bb.instructions
            i = 0
            while i < len(insts):
                inst = insts[i]
                si = inst.sync_info
                tname = type(inst).__name__
                exempt = tname in ("InstNoOp",
                                   "InstEventSemaphore",
                                   "InstUnconditionalBranch", "InstCall",
                                   "InstISA", "InstRegisterMove")
                if (not exempt and si is not None and si.on_wait
                        and len(si.on_wait) > 1):
                    for w in list(si.on_wait[:-1]):
                        nop = mybir.InstNoOp(
                            name=f"wfix-{n}", ins=[], outs=[],
                            engine=inst.engine,
                            sync_info=mybir.SyncInfo(on_wait=[w],
                                                     on_update=[]))
                        insts.insert(i, nop)
                        n += 1
                        i += 1
                    si.on_wait = [si.on_wait[-1]]
                i += 1
    return n


def _build_program(flags):
    (has_bd, has_bq, has_bk, has_bv, has_gq, has_bqln, has_gk, has_bkln,
     has_bu) = flags
    nc = bass.Bass()

    # host-pretransposed x, fp8, block-major: [128, 8*SLAB], col = k*SLAB+s
    x_t8 = nc.declare_dram_parameter("x_t8", [TS, 8 * SLAB], fp8,
                                     isOutput=False)
    x_f32 = nc.declare_dram_parameter("x_f32", [HALF, D], f32, isOutput=False)
    wpack = nc.declare_dram_parameter("wpack", [TS, WPACK_COLS], fp8,
                                      isOutput=False)
    ident = nc.declare_dram_parameter("ident", [TS, TS], bf, isOutput=False)
    hmask = nc.declare_dram_parameter("hmask", [TS, 1], f32, isOutput=False)
    opt = {}
    for name, used, shape in (
        ("bd_c", has_bd, [TS, 4]), ("bq_b", has_bq, [TS, M]),
        ("bk_b", has_bk, [TS, M]), ("bv_b", has_bv, [TS, M]),
        ("gq_b", has_gq, [TS, M]), ("bqln_b", has_bqln, [TS, M]),
        ("gk_b", has_gk, [TS, M]), ("bkln_b", has_bkln, [TS, M]),
        ("bu_b", has_bu, [TS, D]),
    ):
        if used:
            opt[name] = nc.declare_dram_parameter(name, shape, f32,
                                                  isOutput=False)
    y = nc.declare_dram_parameter("y", [HALF, D], f32, isOutput=True)

    with tile.TileContext(nc) as tc, ExitStack() as ctx:
        wpool = ctx.enter_context(tc.tile_pool(name="weights", bufs=1))

        wp_sb = wpool.tile([TS, WPACK_COLS], fp8)
        nc.sync.dma_start(wp_sb[:], wpack[:])
        _off = [0]

        def _wseg(nk, ncols):
            a = _off[0]
            _off[0] += nk * ncols
            return wp_sb[:, a:_off[0]].rearrange("p (k m) -> p k m", k=nk)

        wd_sb = _wseg(8, M)     # [128, 8, 512]: d-blk k -> Wd[d-blk, :]
        wq_sb = _wseg(4, M)
        wk_sb = _wseg(4, M)
        wv_sb = _wseg(4, M)
        w1_sb = _wseg(4, M)
        w2_sb = _wseg(4, M)
        wu_sb = _wseg(4, D)
        tt_sb = _wseg(2, TS)
        id_sb = wpool.tile([TS, TS], bf)
        nc.sync.dma_start(id_sb[:], ident[:])
        hm_sb = wpool.tile([TS, 1], f32)
        nc.sync.dma_start(hm_sb[:], hmask[:])
        opt_sb = {}
        for name, h in opt.items():
            t = wpool.tile([TS, h.shape[1]], f32, tag=name, name=name)
            nc.sync.dma_start(t[:], h[:])
            opt_sb[name] = t

        # SBUF activation pools
        p_xf = ctx.enter_context(tc.tile_pool(name="xf", bufs=9))
        p_xT = ctx.enter_context(tc.tile_pool(name="xT", bufs=2))
        p_hT = ctx.enter_context(tc.tile_pool(name="hT", bufs=2))
        p_kT = ctx.enter_context(tc.tile_pool(name="kT", bufs=2))
        p_a1T = ctx.enter_context(tc.tile_pool(name="a1T", bufs=2))
        p_q = ctx.enter_context(tc.tile_pool(name="q", bufs=18))
        p_v = ctx.enter_context(tc.tile_pool(name="v", bufs=18))
        p_k = ctx.enter_context(tc.tile_pool(name="k", bufs=18))
        p_u = ctx.enter_context(tc.tile_pool(name="u", bufs=18))
        p_rt = ctx.enter_context(tc.tile_pool(name="rt", bufs=3))
        p_y = ctx.enter_context(tc.tile_pool(name="y", bufs=4))
        p_sm = ctx.enter_context(tc.tile_pool(name="sm", bufs=8))
        p_z = ctx.enter_context(tc.tile_pool(name="z", bufs=6))
        # PSUM: pt(2 bf16 banks) + mm(4) + out(2) = 8 banks
        p_pt = ctx.enter_context(tc.tile_pool(name="pt", bufs=2,
                                              space="PSUM"))
        p_mm = ctx.enter_context(tc.tile_pool(name="mm", bufs=4,
                                              space="PSUM"))
        p_out = ctx.enter_context(
            tc.tile_pool(name="out", bufs=1, space="PSUM"))

        def rsqrt_dve(xv, nlan, tag):
            """[128, nlan] f32 = rsqrt(xv) on DVE only (xv consumed)."""
            # seed y0 = bits(MAGIC - bits(x)/2); integer ALU on DVE is not
            # available, so do the bit arithmetic in float value domain
            # (|error| < 128 ulp of bit-space -- Newton absorbs it).
            yv = p_sm.tile([TS, nlan], f32, tag=f"{tag}_y")
            t1 = p_sm.tile([TS, nlan], f32, tag=f"{tag}_t")
            t2 = p_sm.tile([TS, nlan], f32, tag=f"{tag}_t2")
            nc.vector.tensor_copy(t1[:], xv[:].bitcast(u32))
            nc.vector.tensor_scalar(t2[:], t1[:], -0.5, float(MAGIC),
                                    ALU.mult, ALU.add)
            nc.vector.tensor_copy(yv[:].bitcast(u32), t2[:])
            for _ in range(1):  # y <- y*(1.5 - 0.5*x*y^2)
                nc.vector.tensor_mul(t1[:], yv[:], yv[:])
                nc.vector.scalar_tensor_tensor(t1[:], t1[:], -0.5, xv[:],
                                               ALU.mult, ALU.mult)
                nc.vector.tensor_scalar_add(t1[:], t1[:], 1.5)
                nc.vector.tensor_mul(yv[:], yv[:], t1[:])
            return yv

        def ln_pair(z_list, gb_bb):
            """LN over free dim for [zq, zk] (or [zk]) PSUM tiles with one
            shared Newton. The PSUM banks are released after only the
            stats read + a bf16 stash copy (~1us), NOT the full LN chain,
            so the next tile's matmuls get their bank immediately.
            k applies on ACT first (kT needs it); q on DVE."""
            n = len(z_list)
            pools = [p_q, p_k][-n:]
            tags = ["q", "k"][-n:]
            ags, zbs = [], []
            for i, z_ps in enumerate(z_list):
                st = p_sm.tile([TS, 6], f32, tag="bnst")
                nc.vector.bn_stats(st[:], z_ps[:])
                zb = p_z.tile([TS, M], bf, tag=f"z{tags[i]}")
                # split stash copies across engines so each PSUM bank's
                # release isn't queued behind the other's ACT work
                if tags[i] == "q":
                    nc.vector.tensor_copy(zb[:], z_ps[:])
                else:
                    nc.scalar.copy(zb[:], z_ps[:])
                zbs.append(zb)
                ag = p_sm.tile([TS, 2], f32, tag="bnag")
                nc.vector.bn_aggr(ag[:], st[:])
                ags.append(ag)
            xv = p_sm.tile([TS, n], f32, tag="lnx")
            for i, ag in enumerate(ags):
                nc.vector.tensor_scalar_add(xv[:, i:i + 1], ag[:, 1:2],
                                            float(LN_EPS))
            rs = rsqrt_dve(xv, n, "ln")
            outs = []
            for i in reversed(range(n)):
                zb, ag = zbs[i], ags[i]
                gb, bb = gb_bb[i]
                nmr = p_sm.tile([TS, 1], f32, tag=f"nmr{i}")
                nc.vector.scalar_tensor_tensor(nmr[:], ag[:, 0:1], -1.0,
                                               rs[:, i:i + 1],
                                               ALU.mult, ALU.mult)
                o = pools[i].tile([TS, M], bf, tag=tags[i], name=tags[i])
                if i == n - 1:
                    nc.scalar.activation(o[:], zb[:], AF.Identity,
                                         bias=nmr[:], scale=rs[:, i:i + 1])
                else:
                    nc.vector.tensor_scalar(o[:], zb[:], rs[:, i:i + 1],
                                            nmr[:], ALU.mult, ALU.add)
                if gb is not None:
                    nc.vector.tensor_mul(o[:], o[:], gb[:])
                if bb is not None:
                    nc.vector.tensor_add(o[:], o[:], bb[:])
                outs.insert(0, o)
            return outs

        # state carried between phases
        stA = {}           # per-group dict from phase A
        u_prev = [None]

        WMAX = GW * TS

        def phase_a(g0, gn):
            W = gn * TS
            # xT straight from DRAM (host already transposed + fp8-cast)
            xT = p_xT.tile([TS, 8 * WMAX], fp8, tag="xT")
            s0 = g0 * TS
            src = x_t8[:, :].rearrange("p (k s) -> p k s", k=8)[
                :, :, s0:s0 + W]
            dst = xT.rearrange("p (k w) -> p k w", k=8)[:, :, 0:W]
            nc.sync.dma_start(dst, src)

            # hT[m, s] += Wd[d,m].T @ xT[d, s]  (DoubleRow: K=256/mm)
            # s split at 512 (DR moving operand is 2x the out width)
            xTr = xT.rearrange("p (k w) -> p k w", k=8)
            hT = p_hT.tile([TS, 4 * WMAX], fp8, tag="hT")
            for mb in range(4):
                for sh in range(0, W, 512):
                    Wc = min(512, W - sh)
                    acc = p_mm.tile([TS, 4 * TS], f32, tag="mm",
                                    name="hT_ps")
                    for g in range(4):
                        nc.tensor.matmul(
                            acc[:, 0:Wc],
                            wd_sb[:, 2 * g:2 * g + 2, mb * TS:(mb + 1) * TS],
                            xTr[:, 2 * g:2 * g + 2, sh:sh + Wc],
                            start=(g == 0), stop=(g == 3), perf_mode=DR)
                    if has_bd:
                        nc.vector.tensor_scalar_add(
                            acc[:, 0:Wc], acc[:, 0:Wc],
                            opt_sb["bd_c"][:, mb:mb + 1])
                    nc.scalar.copy(
                        hT[:, mb * WMAX + sh:mb * WMAX + sh + Wc],
                        acc[:, 0:Wc])

            qs, vs, ks = [], [], []
            for j in range(gn):
                t_idx = g0 + j
                halo = (t_idx == 0)

                hTr = hT.rearrange("p (k w) -> p k w", k=4)

                def qkv_mm(w3d, name):
                    zp = p_mm.tile([TS, M], f32, tag="mm", name=name)
                    for g in range(2):
                        nc.tensor.matmul(
                            zp[:, 0:M],
                            hTr[:, 2 * g:2 * g + 2, j * TS:(j + 1) * TS],
                            w3d[:, 2 * g:2 * g + 2, 0:M],
                            start=(g == 0), stop=(g == 1), perf_mode=DR)
                    return zp

                zk = qkv_mm(wk_sb, "zk")
                if has_bk:
                    nc.vector.tensor_add(zk[:], zk[:], opt_sb["bk_b"][:])
                if not halo:
                    zq = qkv_mm(wq_sb, "zq")
                    if has_bq:
                        nc.vector.tensor_add(zq[:], zq[:], opt_sb["bq_b"][:])
                    q_sb, k_sb = ln_pair(
                        [zq, zk],
                        [(opt_sb.get("gq_b"), opt_sb.get("bqln_b")),
                         (opt_sb.get("gk_b"), opt_sb.get("bkln_b"))])
                    qs.append(q_sb)
                else:
                    qs.append(None)
                    (k_sb,) = ln_pair(
                        [zk], [(opt_sb.get("gk_b"), opt_sb.get("bkln_b"))])
                ks.append(k_sb)
                zv = qkv_mm(wv_sb, "zv")
                v_sb = p_v.tile([TS, M], bf, tag="v")
                nc.scalar.copy(v_sb[:], zv[:])
                if has_bv:
                    nc.vector.tensor_add(v_sb[:], v_sb[:], opt_sb["bv_b"][:])
                vs.append(v_sb)

            return dict(g0=g0, gn=gn, W=W, hT=hT, qs=qs, vs=vs, ks=ks)

        def phase_a2(st):
            """kT transposes for the whole group (emitted one a-phase
            later, so the LN chain latency is long since hidden)."""
            gn = st["gn"]
            kT = p_kT.tile([TS, 4 * WMAX], fp8, tag="kT")
            for j in range(gn):
                ps = p_pt.tile([TS, 4 * TS], bf, tag="pt", name="ps_k")
                k_sb = st["ks"][j]
                for mb in range(4):
                    nc.tensor.transpose(ps[:, mb * TS:(mb + 1) * TS],
                                        k_sb[:, mb * TS:(mb + 1) * TS],
                                        id_sb[:])
                dst = kT.rearrange("p (k w) -> p k w", k=4)[
                    :, :, j * TS:(j + 1) * TS]
                src = ps[:].rearrange("p (k w) -> p k w", k=4)
                nc.scalar.copy(dst, src)
            st["kT"] = kT

        def phase_b(st):
            g0, gn, W = st["g0"], st["gn"], st["W"]
            xfs = []
            for j in range(gn):
                t_idx = g0 + j
                if t_idx > 0:
                    xf = p_xf.tile([TS, D], f32, tag="xf")
                    nc.sync.dma_start(
                        xf[:], x_f32[(t_idx - 1) * TS:t_idx * TS, :])
                    xfs.append(xf)
                else:
                    xfs.append(None)
            kTr = st["kT"].rearrange("p (k w) -> p k w", k=4)
            # a1T[m1, s] = gelu(W1[m,m1].T @ kT[m, s])
            a1T = p_a1T.tile([TS, 4 * WMAX], fp8, tag="a1T")
            # sh outer: the gelus the first preds need finish mid-phase
            for sh in range(0, W, 512):
                Wc = min(512, W - sh)
                for m1b in range(4):
                    acc = p_mm.tile([TS, 4 * TS], f32, tag="mm",
                                    name="a1_ps")
                    for g in range(2):
                        nc.tensor.matmul(
                            acc[:, 0:Wc],
                            w1_sb[:, 2 * g:2 * g + 2,
                                  m1b * TS:(m1b + 1) * TS],
                            kTr[:, 2 * g:2 * g + 2, sh:sh + Wc],
                            start=(g == 0), stop=(g == 1), perf_mode=DR)
                    nc.scalar.activation(
                        a1T[:, m1b * WMAX + sh:m1b * WMAX + sh + Wc],
                        acc[:, 0:Wc], AF.Gelu_apprx_tanh)

            # breadth-first over the group's tiles so each PE stage's DVE
            # dependencies were produced a stage earlier
            a1Tr = a1T.rearrange("p (k w) -> p k w", k=4)
            preds, us, rtrs, rTs = [], [], [], []
            for j in range(gn):
                pred = p_mm.tile([TS, M], f32, tag="mm", name="pred")
                for g in range(2):
                    nc.tensor.matmul(
                        pred[:, 0:M],
                        a1Tr[:, 2 * g:2 * g + 2, j * TS:(j + 1) * TS],
                        w2_sb[:, 2 * g:2 * g + 2, 0:M],
                        start=(g == 0), stop=(g == 1), perf_mode=DR)
                preds.append(pred)
                u_sb = p_u.tile([TS, M], fp8, tag="u")
                nc.vector.tensor_sub(u_sb[:], st["vs"][j][:], pred[:])
                if g0 + j == 0:
                    nc.vector.tensor_scalar_mul(u_sb[:], u_sb[:],
                                                hm_sb[:, 0:1])
                us.append(u_sb)
            for j in range(gn):
                if g0 + j == 0:
                    u_prev[0] = us[j]
                    rtrs.append(None)
                    continue
                mem = p_mm.tile([TS, M], f32, tag="mm", name="mem")
                nc.tensor.matmul(mem[:], tt_sb[:, 0, :], u_prev[0][:],
                                 start=True, stop=False)
                nc.tensor.matmul(mem[:], tt_sb[:, 1, :], us[j][:],
                                 start=False, stop=True)
                u_prev[0] = us[j]
                rtr = p_rt.tile([TS, M], bf, tag="rtr")
                nc.vector.tensor_mul(rtr[:], st["qs"][j][:], mem[:])
                rtrs.append(rtr)  # 64x-scaled (SCL inside the T matrices)
            for j in range(gn):
                if rtrs[j] is None:
                    rTs.append(None)
                    continue
                ps = p_pt.tile([TS, 4 * TS], bf, tag="pt", name="ps_r")
                for mb in range(4):
                    nc.tensor.transpose(ps[:, mb * TS:(mb + 1) * TS],
                                        rtrs[j][:, mb * TS:(mb + 1) * TS],
                                        id_sb[:])
                rT = p_rt.tile([TS, 4 * TS], fp8, tag="rT")
                nc.scalar.copy(rT[:], ps[:])
                rTs.append(rT)
            for j in range(gn):
                if rTs[j] is None:
                    continue
                t_idx = g0 + j
                rTr = rTs[j].rearrange("p (k w) -> p k w", k=4)
                out_ps = p_out.tile([TS, D], f32)
                for nb in range(2):
                    cols = slice(nb * 512, (nb + 1) * 512)
                    for g in range(2):
                        nc.tensor.matmul(
                            out_ps[:, cols],
                            rTr[:, 2 * g:2 * g + 2, 0:TS],
                            wu_sb[:, 2 * g:2 * g + 2, cols],
                            start=(g == 0), stop=(g == 1), perf_mode=DR)
                y_sb = p_y.tile([TS, D], f32, tag="y")
                # y = x + out/SCL  (out carries the 64x mem scaling)
                nc.vector.scalar_tensor_tensor(
                    y_sb[:], out_ps[:], 1.0 / SCL, xfs[j][:],
                    ALU.mult, ALU.add)
                if has_bu:
                    nc.vector.tensor_add(y_sb[:], y_sb[:], opt_sb["bu_b"][:])
                nc.sync.dma_start(y[(t_idx - 1) * TS:t_idx * TS, :], y_sb[:])

        # software pipeline: A0 K0* A1 B0 K1 A2 B1 ...
        # K(g-1) BEFORE A(g): kT copies land at the ACT queue front (their
        # deps are a full phase old); B(g-1) after A(g) for PE backfill.
        prev = None
        for (g0, gn) in GROUPS:
            if prev is not None:
                phase_a2(prev)
            cur = phase_a(g0, gn)
            if prev is not None:
                phase_b(prev)
            prev = cur
        phase_a2(prev)
        phase_b(prev)

    _fix_matmult_waits(nc)
    return nc


def _prep_inputs(x, Wd, bd, Wq, bq, Wk, bk, Wv, bv, gq, bq_ln, gk, bk_ln,
                 W1, W2, Wu, bu, adaptive_lr, forget_factor):
    """Host-side: flags, decay matrices, per-core slabs, bf16 packing."""
    f = np.float32
    bd, bq, bk, bv, bu = (np.asarray(a, f) for a in (bd, bq, bk, bv, bu))
    gq, bq_ln, gk, bk_ln = (np.asarray(a, f) for a in (gq, bq_ln, gk, bk_ln))
    flags = (bool(bd.any()), bool(bq.any()), bool(bk.any()), bool(bv.any()),
             bool((gq != 1).any()), bool(bq_ln.any()),
             bool((gk != 1).any()), bool(bk_ln.any()), bool(bu.any()))

    g = 1.0 / (1.0 + np.exp(-np.float64(forget_factor)))
    lr = np.float64(adaptive_lr)
    t_idx = np.arange(TS)
    lag_cur = t_idx[:, None] - t_idx[None, :]
    Tcur = np.where(lag_cur >= 0, g ** np.maximum(lag_cur, 0), 0.0) * lr * SCL
    lag_prev = t_idx[:, None] + TS - t_idx[None, :]
    Tprev = (g ** lag_prev) * lr * SCL
    TT = np.concatenate([Tprev, Tcur], axis=1).T.astype(f)  # [256, 128]

    def seg(w):
        w = np.asarray(w, f)          # [K, N] -> [128, nk*N]
        nk = w.shape[0] // TS
        return w.reshape(nk, TS, w.shape[1]).transpose(1, 0, 2).reshape(TS, -1)

    wpack = np.ascontiguousarray(np.concatenate(
        [seg(w) for w in (Wd, Wq, Wk, Wv, W1, W2, Wu, TT)],
        axis=1)).astype(np_fp8)
    common = {
        "wpack": wpack,
        "ident": np.eye(TS, dtype=f).astype(np_bf16),
    }
    names = ("bd_c", "bq_b", "bk_b", "bv_b", "gq_b", "bqln_b", "gk_b",
             "bkln_b", "bu_b")
    vecs = (bd, bq, bk, bv, gq, bq_ln, gk, bk_ln, bu)
    for name, used, vec in zip(names, flags, vecs):
        if not used:
            continue
        if name == "bd_c":
            common[name] = np.ascontiguousarray(
                vec.reshape(4, TS).T, f)      # [128, 4]: col mb = bd block
        else:
            common[name] = np.ascontiguousarray(
                np.broadcast_to(vec, (TS, vec.shape[0])), f)

    x = np.asarray(x, f)
    in_maps = []
    for c in range(N_CORES):
        b, sh = c // 2, c % 2
        if sh == 0:
            haloblk = np.zeros((TS, D), f)
            hm = np.zeros((TS, 1), f)
        else:
            haloblk = x[b, HALF - TS:HALF]
            hm = np.ones((TS, 1), f)
        slab = np.concatenate([haloblk, x[b, sh * HALF:(sh + 1) * HALF]],
                              axis=0)
        m = dict(common)
        # [SLAB, D] -> transpose -> [8, 128, SLAB] -> [128, 8*SLAB] fp8
        xt = np.ascontiguousarray(slab.T).reshape(8, TS, SLAB)
        m["x_t8"] = np.ascontiguousarray(
            xt.transpose(1, 0, 2).reshape(TS, 8 * SLAB)).astype(np_fp8)
        m["x_f32"] = np.ascontiguousarray(x[b, sh * HALF:(sh + 1) * HALF])
        m["hmask"] = hm
        in_maps.append(m)
    return flags, in_maps


def kernel(**inputs):
    global LAST_RESULTS
    flags, in_maps = _prep_inputs(**inputs)
    if flags not in _PROG_CACHE:
        _PROG_CACHE[flags] = _build_program(flags)
    nc = _PROG_CACHE[flags]

    res = run_bass_kernel_spmd(nc, in_maps, list(range(N_CORES)),
                               trace=TRACE, trace_kwargs=TRACE_KWARGS)
    LAST_RESULTS = res

    out = np.empty((B, S, D), np.float32)
    for c in range(N_CORES):
        b, sh = c // 2, c % 2
        out[b, sh * HALF:(sh + 1) * HALF] = res.results[c]["y"]
    return out


# revision 74
# speedup vs baseline: 3.7377x; 1.0518x over previous
"""Trainium2 Bass kernel for AdvancedNeuralMemory (B=4, S=8192, D=1024, M=512).

Math
----
s_t = g*s_{t-1} + u_t with scalar g = sigmoid(forget_factor) ~ 0.525.
g^129 < fp32 eps, so mem for a 128-row tile is exactly
    mem_i = Tprev.T @ u_{i-1} + Tcur.T @ u_i
with host-precomputed decay-Toeplitz matrices (adaptive_lr folded in).
Sequential scan -> pure matmuls; 8 cores = (batch 0..3) x (seq half 0..1),
each works a [4096,1024] slab + one 128-row halo tile. No cross-core comm.

V2 design (from trace analysis of the fp32 baseline @ 978us):
 * bf16 operands everywhere on the PE (fp32 PSUM accumulate). Inputs are
   cast host-side; residual add uses an fp32 copy of x; y stays fp32.
 * hT and a1T computed directly in transposed orientation with the
   *weights* as the stationary operand over 512-row macro-tiles:
   kills the h- and a1- PE transposes and their PSUM->SBUF copies.
 * LN inv-std via DVE Newton rsqrt (bit-hack seed): the scalar engine
   never touches the Sqrt table set, so the Gelu table stays resident
   (the baseline lost ~5.4us/tile to ACT_TABLE_LOAD thrash).
 * macro-level software pipeline: A(g) = x/xT/hT/qkv/LN/kT,
   B(g) = a1T/pred/u/mem/rtr/rT/out/y, issued A0 A1 B0 A2 B1 ... so the
   PE never waits on the LN->kT chain of the current group.
"""

import sys
import os

for _p in ("/opt/trn_rl_repo",):
    if _p not in sys.path and os.path.isdir(_p):
        sys.path.insert(0, _p)

from contextlib import ExitStack

import numpy as np
import ml_dtypes

import concourse.bass as bass
import concourse.mybir as mybir
import concourse.tile as tile
from concourse.bass_utils import run_bass_kernel_spmd

B, S, D, M = 4, 8192, 1024, 512
HALF = S // 2          # rows per core
TS = 128               # s-tile rows
NT = HALF // TS        # compute tiles per core (32)
SLAB = HALF + TS       # slab rows incl. halo tile
LN_EPS = 1e-5
N_CORES = 8
GW = 8                 # tiles per macro-group
# groups of tile indices: [0..3],[4..7],...,[28..31],[32]
GROUPS = [(g, min(GW, NT + 1 - g)) for g in range(0, NT + 1, GW)]

f32 = mybir.dt.float32
bf = mybir.dt.bfloat16
fp8 = mybir.dt.float8e4
u32 = mybir.dt.uint32
AF = mybir.ActivationFunctionType
ALU = mybir.AluOpType
DR = mybir.MatmulPerfMode.DoubleRow
np_bf16 = ml_dtypes.bfloat16
np_fp8 = ml_dtypes.float8_e4m3
SCL = 64.0            # mem scaling so fp8 operands sit in normal range

# packed bf16 weights: wd(8*512) wq/wk/wv/w1/w2(4*512) wu(4*1024) tt(2*128)
WPACK_COLS = 8 * M + 5 * 4 * M + 4 * D + 2 * TS

TRACE = False
TRACE_KWARGS = {}
LAST_RESULTS = None

_PROG_CACHE = {}

MAGIC = 0x5F3759DF


def _fix_matmult_waits(nc):
    """Walrus allows only one sync-wait on a (fused-ldweights) Matmult.
    Move surplus waits onto an inserted NoOp on the same engine."""
    n = 0
    for f in nc.m.functions:
        for bb in f.blocks:
            insts = bb.instructions
            i = 0
            while i < len(insts):
                inst = insts[i]
                si = inst.sync_info
                tname = type(inst).__name__
                exempt = tname in ("InstNoOp",
                                   "InstEventSemaphore",
                                   "InstUnconditionalBranch", "InstCall",
                                   "InstISA", "InstRegisterMove")
                if (not exempt and si is not None and si.on_wait
                        and len(si.on_wait) > 1):
                    for w in list(si.on_wait[:-1]):
                        nop = mybir.InstNoOp(
                            name=f"wfix-{n}", ins=[], outs=[],
                            engine=inst.engine,
                            sync_info=mybir.SyncInfo(on_wait=[w],
                                                     on_update=[]))
                        insts.insert(i, nop)
                        n += 1
                        i += 1
                    si.on_wait = [si.on_wait[-1]]
                i += 1
    return n


def _build_program(flags):
    (has_bd, has_bq, has_bk, has_bv, has_gq, has_bqln, has_gk, has_bkln,
     has_bu) = flags
    nc = bass.Bass()

    # host-pretransposed x, fp8, block-major: [128, 8*SLAB], col = k*SLAB+s
    x_t8 = nc.declare_dram_parameter("x_t8", [TS, 8 * SLAB], fp8,
                                     isOutput=False)
    x_f32 = nc.declare_dram_parameter("x_f32", [HALF, D], f32, isOutput=False)
    wpack = nc.declare_dram_parameter("wpack", [TS, WPACK_COLS], fp8,
                                      isOutput=False)
    ident = nc.declare_dram_parameter("ident", [TS, TS], bf, isOutput=False)
    hmask = nc.declare_dram_parameter("hmask", [TS, 1], f32, isOutput=False)
    opt = {}
    for name, used, shape in (
        ("bd_c", has_bd, [TS, 4]), ("bq_b", has_bq, [TS, M]),
        ("bk_b", has_bk, [TS, M]), ("bv_b", has_bv, [TS, M]),
        ("gq_b", has_gq, [TS, M]), ("bqln_b", has_bqln, [TS, M]),
        ("gk_b", has_gk, [TS, M]), ("bkln_b", has_bkln, [TS, M]),
        ("bu_b", has_bu, [TS, D]),
    ):
        if used:
            opt[name] = nc.declare_dram_parameter(name, shape, f32,
                                                  isOutput=False)
    y = nc.declare_dram_parameter("y", [HALF, D], f32, isOutput=True)

    with tile.TileContext(nc) as tc, ExitStack() as ctx:
        wpool = ctx.enter_context(tc.tile_pool(name="weights", bufs=1))

        wp_sb = wpool.tile([TS, WPACK_COLS], fp8)
        nc.sync.dma_start(wp_sb[:], wpack[:])
        _off = [0]

        def _wseg(nk, ncols):
            a = _off[0]
            _off[0] += nk * ncols
            return wp_sb[:, a:_off[0]].rearrange("p (k m) -> p k m", k=nk)

        wd_sb = _wseg(8, M)     # [128, 8, 512]: d-blk k -> Wd[d-blk, :]
        wq_sb = _wseg(4, M)
        wk_sb = _wseg(4, M)
        wv_sb = _wseg(4, M)
        w1_sb = _wseg(4, M)
        w2_sb = _wseg(4, M)
        wu_sb = _wseg(4, D)
        tt_sb = _wseg(2, TS)
        id_sb = wpool.tile([TS, TS], bf)
        nc.sync.dma_start(id_sb[:], ident[:])
        hm_sb = wpool.tile([TS, 1], f32)
        nc.sync.dma_start(hm_sb[:], hmask[:])
        opt_sb = {}
        for name, h in opt.items():
            t = wpool.tile([TS, h.shape[1]], f32, tag=name, name=name)
            nc.sync.dma_start(t[:], h[:])
            opt_sb[name] = t

        # SBUF activation pools
        p_xf = ctx.enter_context(tc.tile_pool(name="xf", bufs=9))
        p_xT = ctx.enter_context(tc.tile_pool(name="xT", bufs=2))
        p_hT = ctx.enter_context(tc.tile_pool(name="hT", bufs=2))
        p_kT = ctx.enter_context(tc.tile_pool(name="kT", bufs=2))
        p_a1T = ctx.enter_context(tc.tile_pool(name="a1T", bufs=2))
        p_q = ctx.enter_context(tc.tile_pool(name="q", bufs=18))
        p_v = ctx.enter_context(tc.tile_pool(name="v", bufs=18))
        p_k = ctx.enter_context(tc.tile_pool(name="k", bufs=18))
        p_u = ctx.enter_context(tc.tile_pool(name="u", bufs=18))
        p_rt = ctx.enter_context(tc.tile_pool(name="rt", bufs=3))
        p_y = ctx.enter_context(tc.tile_pool(name="y", bufs=4))
        p_sm = ctx.enter_context(tc.tile_pool(name="sm", bufs=8))
        p_z = ctx.enter_context(tc.tile_pool(name="z", bufs=6))
        # PSUM: pt(2 bf16 banks) + mm(4) + out(2) = 8 banks
        p_pt = ctx.enter_context(tc.tile_pool(name="pt", bufs=2,
                                              space="PSUM"))
        p_mm = ctx.enter_context(tc.tile_pool(name="mm", bufs=4,
                                              space="PSUM"))
        p_out = ctx.enter_context(
            tc.tile_pool(name="out", bufs=2, space="PSUM"))

        def rsqrt_dve(xv, nlan, tag):
            """[128, nlan] f32 = rsqrt(xv) on DVE only (xv consumed)."""
            # seed y0 = bits(MAGIC - bits(x)/2); integer ALU on DVE is not
            # available, so do the bit arithmetic in float value domain
            # (|error| < 128 ulp of bit-space -- Newton absorbs it).
            yv = p_sm.tile([TS, nlan], f32, tag=f"{tag}_y")
            t1 = p_sm.tile([TS, nlan], f32, tag=f"{tag}_t")
            t2 = p_sm.tile([TS, nlan], f32, tag=f"{tag}_t2")
            nc.vector.tensor_copy(t1[:], xv[:].bitcast(u32))
            nc.vector.tensor_scalar(t2[:], t1[:], -0.5, float(MAGIC),
                                    ALU.mult, ALU.add)
            nc.vector.tensor_copy(yv[:].bitcast(u32), t2[:])
            for _ in range(1):  # y <- y*(1.5 - 0.5*x*y^2)
                nc.vector.tensor_mul(t1[:], yv[:], yv[:])
                nc.vector.scalar_tensor_tensor(t1[:], t1[:], -0.5, xv[:],
                                               ALU.mult, ALU.mult)
                nc.vector.tensor_scalar_add(t1[:], t1[:], 1.5)
                nc.vector.tensor_mul(yv[:], yv[:], t1[:])
            return yv

        def ln_pair(z_list, gb_bb):
            """LN over free dim for [zq, zk] (or [zk]) PSUM tiles with one
            shared Newton. The PSUM banks are released after only the
            stats read + a bf16 stash copy (~1us), NOT the full LN chain,
            so the next tile's matmuls get their bank immediately.
            k applies on ACT first (kT needs it); q on DVE."""
            n = len(z_list)
            pools = [p_q, p_k][-n:]
            tags = ["q", "k"][-n:]
            ags, zbs = [], []
            for i, z_ps in enumerate(z_list):
                st = p_sm.tile([TS, 6], f32, tag="bnst")
                nc.vector.bn_stats(st[:], z_ps[:])
                zb = p_z.tile([TS, M], bf, tag=f"z{tags[i]}")
                nc.scalar.copy(zb[:], z_ps[:])
                zbs.append(zb)
                ag = p_sm.tile([TS, 2], f32, tag="bnag")
                nc.vector.bn_aggr(ag[:], st[:])
                ags.append(ag)
            xv = p_sm.tile([TS, n], f32, tag="lnx")
            for i, ag in enumerate(ags):
                nc.vector.tensor_scalar_add(xv[:, i:i + 1], ag[:, 1:2],
                                            float(LN_EPS))
            rs = rsqrt_dve(xv, n, "ln")
            outs = []
            for i in reversed(range(n)):
                zb, ag = zbs[i], ags[i]
                gb, bb = gb_bb[i]
                nmr = p_sm.tile([TS, 1], f32, tag=f"nmr{i}")
                nc.vector.scalar_tensor_tensor(nmr[:], ag[:, 0:1], -1.0,
                                               rs[:, i:i + 1],
                                               ALU.mult, ALU.mult)
                o = pools[i].tile([TS, M], bf, tag=tags[i], name=tags[i])
                if i == n - 1:
                    nc.scalar.activation(o[:], zb[:], AF.Identity,
                                         bias=nmr[:], scale=rs[:, i:i + 1])
                else:
                    nc.vector.tensor_scalar(o[:], zb[:], rs[:, i:i + 1],
                                            nmr[:], ALU.mult, ALU.add)
                if gb is not None:
                    nc.vector.tensor_mul(o[:], o[:], gb[:])
                if bb is not None:
                    nc.vector.tensor_add(o[:], o[:], bb[:])
                outs.insert(0, o)
            return outs

        # state carried between phases
        stA = {}           # per-group dict from phase A
        u_prev = [None]

        WMAX = GW * TS

        def phase_a(g0, gn):
            W = gn * TS
            # xT straight from DRAM (host already transposed + fp8-cast)
            xT = p_xT.tile([TS, 8 * WMAX], fp8, tag="xT")
            s0 = g0 * TS
            src = x_t8[:, :].rearrange("p (k s) -> p k s", k=8)[
                :, :, s0:s0 + W]
            dst = xT.rearrange("p (k w) -> p k w", k=8)[:, :, 0:W]
            nc.sync.dma_start(dst, src)

            # hT[m, s] += Wd[d,m].T @ xT[d, s]  (DoubleRow: K=256/mm)
            # s split at 512 (DR moving operand is 2x the out width)
            xTr = xT.rearrange("p (k w) -> p k w", k=8)
            hT = p_hT.tile([TS, 4 * WMAX], fp8, tag="hT")
            for mb in range(4):
                for sh in range(0, W, 512):
                    Wc = min(512, W - sh)
                    acc = p_mm.tile([TS, 4 * TS], f32, tag="mm",
                                    name="hT_ps")
                    for g in range(4):
                        nc.tensor.matmul(
                            acc[:, 0:Wc],
                            wd_sb[:, 2 * g:2 * g + 2, mb * TS:(mb + 1) * TS],
                            xTr[:, 2 * g:2 * g + 2, sh:sh + Wc],
                            start=(g == 0), stop=(g == 3), perf_mode=DR)
                    if has_bd:
                        nc.vector.tensor_scalar_add(
                            acc[:, 0:Wc], acc[:, 0:Wc],
                            opt_sb["bd_c"][:, mb:mb + 1])
                    nc.scalar.copy(
                        hT[:, mb * WMAX + sh:mb * WMAX + sh + Wc],
                        acc[:, 0:Wc])

            qs, vs, ks = [], [], []
            for j in range(gn):
                t_idx = g0 + j
                halo = (t_idx == 0)

                hTr = hT.rearrange("p (k w) -> p k w", k=4)

                def qkv_mm(w3d, name):
                    zp = p_mm.tile([TS, M], f32, tag="mm", name=name)
                    for g in range(2):
                        nc.tensor.matmul(
                            zp[:, 0:M],
                            hTr[:, 2 * g:2 * g + 2, j * TS:(j + 1) * TS],
                            w3d[:, 2 * g:2 * g + 2, 0:M],
                            start=(g == 0), stop=(g == 1), perf_mode=DR)
                    return zp

                zk = qkv_mm(wk_sb, "zk")
                if has_bk:
                    nc.vector.tensor_add(zk[:], zk[:], opt_sb["bk_b"][:])
                if not halo:
                    zq = qkv_mm(wq_sb, "zq")
                    if has_bq:
                        nc.vector.tensor_add(zq[:], zq[:], opt_sb["bq_b"][:])
                    q_sb, k_sb = ln_pair(
                        [zq, zk],
                        [(opt_sb.get("gq_b"), opt_sb.get("bqln_b")),
                         (opt_sb.get("gk_b"), opt_sb.get("bkln_b"))])
                    qs.append(q_sb)
                else:
                    qs.append(None)
                    (k_sb,) = ln_pair(
                        [zk], [(opt_sb.get("gk_b"), opt_sb.get("bkln_b"))])
                ks.append(k_sb)
                zv = qkv_mm(wv_sb, "zv")
                v_sb = p_v.tile([TS, M], bf, tag="v")
                nc.scalar.copy(v_sb[:], zv[:])
                if has_bv:
                    nc.vector.tensor_add(v_sb[:], v_sb[:], opt_sb["bv_b"][:])
                vs.append(v_sb)

            return dict(g0=g0, gn=gn, W=W, hT=hT, qs=qs, vs=vs, ks=ks)

        def phase_a2(st):
            """kT transposes for the whole group (emitted one a-phase
            later, so the LN chain latency is long since hidden)."""
            gn = st["gn"]
            kT = p_kT.tile([TS, 4 * WMAX], fp8, tag="kT")
            for j in range(gn):
                ps = p_pt.tile([TS, 4 * TS], bf, tag="pt", name="ps_k")
                k_sb = st["ks"][j]
                for mb in range(4):
                    nc.tensor.transpose(ps[:, mb * TS:(mb + 1) * TS],
                                        k_sb[:, mb * TS:(mb + 1) * TS],
                                        id_sb[:])
                dst = kT.rearrange("p (k w) -> p k w", k=4)[
                    :, :, j * TS:(j + 1) * TS]
                src = ps[:].rearrange("p (k w) -> p k w", k=4)
                nc.scalar.copy(dst, src)
            st["kT"] = kT

        def phase_b(st):
            g0, gn, W = st["g0"], st["gn"], st["W"]
            xfs = []
            for j in range(gn):
                t_idx = g0 + j
                if t_idx > 0:
                    xf = p_xf.tile([TS, D], f32, tag="xf")
                    nc.sync.dma_start(
                        xf[:], x_f32[(t_idx - 1) * TS:t_idx * TS, :])
                    xfs.append(xf)
                else:
                    xfs.append(None)
            kTr = st["kT"].rearrange("p (k w) -> p k w", k=4)
            # a1T[m1, s] = gelu(W1[m,m1].T @ kT[m, s])
            a1T = p_a1T.tile([TS, 4 * WMAX], fp8, tag="a1T")
            for m1b in range(4):
                for sh in range(0, W, 512):
                    Wc = min(512, W - sh)
                    acc = p_mm.tile([TS, 4 * TS], f32, tag="mm",
                                    name="a1_ps")
                    for g in range(2):
                        nc.tensor.matmul(
                            acc[:, 0:Wc],
                            w1_sb[:, 2 * g:2 * g + 2,
                                  m1b * TS:(m1b + 1) * TS],
                            kTr[:, 2 * g:2 * g + 2, sh:sh + Wc],
                            start=(g == 0), stop=(g == 1), perf_mode=DR)
                    nc.scalar.activation(
                        a1T[:, m1b * WMAX + sh:m1b * WMAX + sh + Wc],
                        acc[:, 0:Wc], AF.Gelu_apprx_tanh)

            # breadth-first over the group's tiles so each PE stage's DVE
            # dependencies were produced a stage earlier
            a1Tr = a1T.rearrange("p (k w) -> p k w", k=4)
            preds, us, rtrs, rTs = [], [], [], []
            for j in range(gn):
                pred = p_mm.tile([TS, M], f32, tag="mm", name="pred")
                for g in range(2):
                    nc.tensor.matmul(
                        pred[:, 0:M],
                        a1Tr[:, 2 * g:2 * g + 2, j * TS:(j + 1) * TS],
                        w2_sb[:, 2 * g:2 * g + 2, 0:M],
                        start=(g == 0), stop=(g == 1), perf_mode=DR)
                preds.append(pred)
                u_sb = p_u.tile([TS, M], fp8, tag="u")
                nc.vector.tensor_sub(u_sb[:], st["vs"][j][:], pred[:])
                if g0 + j == 0:
                    nc.vector.tensor_scalar_mul(u_sb[:], u_sb[:],
                                                hm_sb[:, 0:1])
                us.append(u_sb)
            for j in range(gn):
                if g0 + j == 0:
                    u_prev[0] = us[j]
                    rtrs.append(None)
                    continue
                mem = p_mm.tile([TS, M], f32, tag="mm", name="mem")
                nc.tensor.matmul(mem[:], tt_sb[:, 0, :], u_prev[0][:],
                                 start=True, stop=False)
                nc.tensor.matmul(mem[:], tt_sb[:, 1, :], us[j][:],
                                 start=False, stop=True)
                u_prev[0] = us[j]
                rtr = p_rt.tile([TS, M], bf, tag="rtr")
                nc.vector.tensor_mul(rtr[:], st["qs"][j][:], mem[:])
                rtrs.append(rtr)  # 64x-scaled (SCL inside the T matrices)
            for j in range(gn):
                if rtrs[j] is None:
                    rTs.append(None)
                    continue
                ps = p_pt.tile([TS, 4 * TS], bf, tag="pt", name="ps_r")
                for mb in range(4):
                    nc.tensor.transpose(ps[:, mb * TS:(mb + 1) * TS],
                                        rtrs[j][:, mb * TS:(mb + 1) * TS],
                                        id_sb[:])
                rT = p_rt.tile([TS, 4 * TS], fp8, tag="rT")
                nc.scalar.copy(rT[:], ps[:])
                rTs.append(rT)
            for j in range(gn):
                if rTs[j] is None:
                    continue
                t_idx = g0 + j
                rTr = rTs[j].rearrange("p (k w) -> p k w", k=4)
                # out in two 1-bank halves (p_out bufs=2): the next half's
                # matmuls only wait for a half-sized y, halving the
                # PE<->DVE lockstep granularity at the same PSUM budget
                for nb in range(2):
                    cols = slice(nb * 512, (nb + 1) * 512)
                    out_ps = p_out.tile([TS, 512], f32, tag="out")
                    for g in range(2):
                        nc.tensor.matmul(
                            out_ps[:],
                            rTr[:, 2 * g:2 * g + 2, 0:TS],
                            wu_sb[:, 2 * g:2 * g + 2, cols],
                            start=(g == 0), stop=(g == 1), perf_mode=DR)
                    y_sb = p_y.tile([TS, 512], f32, tag="y")
                    # y = x + out/SCL  (out carries the 64x mem scaling)
                    nc.vector.scalar_tensor_tensor(
                        y_sb[:], out_ps[:], 1.0 / SCL, xfs[j][:, cols],
                        ALU.mult, ALU.add)
                    if has_bu:
                        nc.vector.tensor_add(y_sb[:], y_sb[:],
                                             opt_sb["bu_b"][:, cols])
                    nc.sync.dma_start(y[(t_idx - 1) * TS:t_idx * TS, cols],
                                      y_sb[:])

        # software pipeline: A0 K0* A1 B0 K1 A2 B1 ...
        # K(g-1) BEFORE A(g): kT copies land at the ACT queue front (their
        # deps are a full phase old); B(g-1) after A(g) for PE backfill.
        prev = None
        for (g0, gn) in GROUPS:
            if prev is not None:
                phase_a2(prev)
            cur = phase_a(g0, gn)
            if prev is not None:
                phase_b(prev)
            prev = cur
        phase_a2(prev)
        phase_b(prev)

    _fix_matmult_waits(nc)
    return nc


def _prep_inputs(x, Wd, bd, Wq, bq, Wk, bk, Wv, bv, gq, bq_ln, gk, bk_ln,
                 W1, W2, Wu, bu, adaptive_lr, forget_factor):
    """Host-side: flags, decay matrices, per-core slabs, bf16 packing."""
    f = np.float32
    bd, bq, bk, bv, bu = (np.asarray(a, f) for a in (bd, bq, bk, bv, bu))
    gq, bq_ln, gk, bk_ln = (np.asarray(a, f) for a in (gq, bq_ln, gk, bk_ln))
    flags = (bool(bd.any()), bool(bq.any()), bool(bk.any()), bool(bv.any()),
             bool((gq != 1).any()), bool(bq_ln.any()),
             bool((gk != 1).any()), bool(bk_ln.any()), bool(bu.any()))

    g = 1.0 / (1.0 + np.exp(-np.float64(forget_factor)))
    lr = np.float64(adaptive_lr)
    t_idx = np.arange(TS)
    lag_cur = t_idx[:, None] - t_idx[None, :]
    Tcur = np.where(lag_cur >= 0, g ** np.maximum(lag_cur, 0), 0.0) * lr * SCL
    lag_prev = t_idx[:, None] + TS - t_idx[None, :]
    Tprev = (g ** lag_prev) * lr * SCL
    TT = np.concatenate([Tprev, Tcur], axis=1).T.astype(f)  # [256, 128]

    def seg(w):
        w = np.asarray(w, f)          # [K, N] -> [128, nk*N]
        nk = w.shape[0] // TS
        return w.reshape(nk, TS, w.shape[1]).transpose(1, 0, 2).reshape(TS, -1)

    wpack = np.ascontiguousarray(np.concatenate(
        [seg(w) for w in (Wd, Wq, Wk, Wv, W1, W2, Wu, TT)],
        axis=1)).astype(np_fp8)
    common = {
        "wpack": wpack,
        "ident": np.eye(TS, dtype=f).astype(np_bf16),
    }
    names = ("bd_c", "bq_b", "bk_b", "bv_b", "gq_b", "bqln_b", "gk_b",
             "bkln_b", "bu_b")
    vecs = (bd, bq, bk, bv, gq, bq_ln, gk, bk_ln, bu)
    for name, used, vec in zip(names, flags, vecs):
        if not used:
            continue
        if name == "bd_c":
            common[name] = np.ascontiguousarray(
                vec.reshape(4, TS).T, f)      # [128, 4]: col mb = bd block
        else:
            common[name] = np.ascontiguousarray(
                np.broadcast_to(vec, (TS, vec.shape[0])), f)

    x = np.asarray(x, f)
    in_maps = []
    for c in range(N_CORES):
        b, sh = c // 2, c % 2
        if sh == 0:
            haloblk = np.zeros((TS, D), f)
            hm = np.zeros((TS, 1), f)
        else:
            haloblk = x[b, HALF - TS:HALF]
            hm = np.ones((TS, 1), f)
        slab = np.concatenate([haloblk, x[b, sh * HALF:(sh + 1) * HALF]],
                              axis=0)
        m = dict(common)
        # [SLAB, D] -> transpose -> [8, 128, SLAB] -> [128, 8*SLAB] fp8
        xt = np.ascontiguousarray(slab.T).reshape(8, TS, SLAB)
        m["x_t8"] = np.ascontiguousarray(
            xt.transpose(1, 0, 2).reshape(TS, 8 * SLAB)).astype(np_fp8)
        m["x_f32"] = np.ascontiguousarray(x[b, sh * HALF:(sh + 1) * HALF])
        m["hmask"] = hm
        in_maps.append(m)
    return flags, in_maps


def kernel(**inputs):
    global LAST_RESULTS
    flags, in_maps = _prep_inputs(**inputs)
    if flags not in _PROG_CACHE:
        _PROG_CACHE[flags] = _build_program(flags)
    nc = _PROG_CACHE[flags]

    res = run_bass_kernel_spmd(nc, in_maps, list(range(N_CORES)),
                               trace=TRACE, trace_kwargs=TRACE_KWARGS)
    LAST_RESULTS = res

    out = np.empty((B, S, D), np.float32)
    for c in range(N_CORES):
        b, sh = c // 2, c % 2
        out[b, sh * HALF:(sh + 1) * HALF] = res.results[c]["y"]
    return out


# revision 75
# speedup vs baseline: 3.7475x; 1.0026x over previous
"""Trainium2 Bass kernel for AdvancedNeuralMemory (B=4, S=8192, D=1024, M=512).

Math
----
s_t = g*s_{t-1} + u_t with scalar g = sigmoid(forget_factor) ~ 0.525.
g^129 < fp32 eps, so mem for a 128-row tile is exactly
    mem_i = Tprev.T @ u_{i-1} + Tcur.T @ u_i
with host-precomputed decay-Toeplitz matrices (adaptive_lr folded in).
Sequential scan -> pure matmuls; 8 cores = (batch 0..3) x (seq half 0..1),
each works a [4096,1024] slab + one 128-row halo tile. No cross-core comm.

V2 design (from trace analysis of the fp32 baseline @ 978us):
 * bf16 operands everywhere on the PE (fp32 PSUM accumulate). Inputs are
   cast host-side; residual add uses an fp32 copy of x; y stays fp32.
 * hT and a1T computed directly in transposed orientation with the
   *weights* as the stationary operand over 512-row macro-tiles:
   kills the h- and a1- PE transposes and their PSUM->SBUF copies.
 * LN inv-std via DVE Newton rsqrt (bit-hack seed): the scalar engine
   never touches the Sqrt table set, so the Gelu table stays resident
   (the baseline lost ~5.4us/tile to ACT_TABLE_LOAD thrash).
 * macro-level software pipeline: A(g) = x/xT/hT/qkv/LN/kT,
   B(g) = a1T/pred/u/mem/rtr/rT/out/y, issued A0 A1 B0 A2 B1 ... so the
   PE never waits on the LN->kT chain of the current group.
"""

import sys
import os

for _p in ("/opt/trn_rl_repo",):
    if _p not in sys.path and os.path.isdir(_p):
        sys.path.insert(0, _p)

from contextlib import ExitStack

import numpy as np
import ml_dtypes

import concourse.bass as bass
import concourse.mybir as mybir
import concourse.tile as tile
from concourse.bass_utils import run_bass_kernel_spmd

B, S, D, M = 4, 8192, 1024, 512
HALF = S // 2          # rows per core
TS = 128               # s-tile rows
NT = HALF // TS        # compute tiles per core (32)
SLAB = HALF + TS       # slab rows incl. halo tile
LN_EPS = 1e-5
N_CORES = 8
GW = 9                 # max tiles per macro-group (sizes 8,8,8,9)
# merge the single-tile tail into the last group: one less pipeline drain
GROUPS = [(0, 8), (8, 8), (16, 8), (24, 9)]

f32 = mybir.dt.float32
bf = mybir.dt.bfloat16
fp8 = mybir.dt.float8e4
u32 = mybir.dt.uint32
AF = mybir.ActivationFunctionType
ALU = mybir.AluOpType
DR = mybir.MatmulPerfMode.DoubleRow
np_bf16 = ml_dtypes.bfloat16
np_fp8 = ml_dtypes.float8_e4m3
SCL = 64.0            # mem scaling so fp8 operands sit in normal range

# packed bf16 weights: wd(8*512) wq/wk/wv/w1/w2(4*512) wu(4*1024) tt(2*128)
WPACK_COLS = 8 * M + 5 * 4 * M + 4 * D + 2 * TS

TRACE = False
TRACE_KWARGS = {}
LAST_RESULTS = None

_PROG_CACHE = {}

MAGIC = 0x5F3759DF


def _fix_matmult_waits(nc):
    """Walrus allows only one sync-wait on a (fused-ldweights) Matmult.
    Move surplus waits onto an inserted NoOp on the same engine."""
    n = 0
    for f in nc.m.functions:
        for bb in f.blocks:
            insts = bb.instructions
            i = 0
            while i < len(insts):
                inst = insts[i]
                si = inst.sync_info
                tname = type(inst).__name__
                exempt = tname in ("InstNoOp",
                                   "InstEventSemaphore",
                                   "InstUnconditionalBranch", "InstCall",
                                   "InstISA", "InstRegisterMove")
                if (not exempt and si is not None and si.on_wait
                        and len(si.on_wait) > 1):
                    for w in list(si.on_wait[:-1]):
                        nop = mybir.InstNoOp(
                            name=f"wfix-{n}", ins=[], outs=[],
                            engine=inst.engine,
                            sync_info=mybir.SyncInfo(on_wait=[w],
                                                     on_update=[]))
                        insts.insert(i, nop)
                        n += 1
                        i += 1
                    si.on_wait = [si.on_wait[-1]]
                i += 1
    return n


def _build_program(flags):
    (has_bd, has_bq, has_bk, has_bv, has_gq, has_bqln, has_gk, has_bkln,
     has_bu) = flags
    nc = bass.Bass()

    # host-pretransposed x, fp8, block-major: [128, 8*SLAB], col = k*SLAB+s
    x_t8 = nc.declare_dram_parameter("x_t8", [TS, 8 * SLAB], fp8,
                                     isOutput=False)
    x_f32 = nc.declare_dram_parameter("x_f32", [HALF, D], f32, isOutput=False)
    wpack = nc.declare_dram_parameter("wpack", [TS, WPACK_COLS], fp8,
                                      isOutput=False)
    ident = nc.declare_dram_parameter("ident", [TS, TS], bf, isOutput=False)
    hmask = nc.declare_dram_parameter("hmask", [TS, 1], f32, isOutput=False)
    opt = {}
    for name, used, shape in (
        ("bd_c", has_bd, [TS, 4]), ("bq_b", has_bq, [TS, M]),
        ("bk_b", has_bk, [TS, M]), ("bv_b", has_bv, [TS, M]),
        ("gq_b", has_gq, [TS, M]), ("bqln_b", has_bqln, [TS, M]),
        ("gk_b", has_gk, [TS, M]), ("bkln_b", has_bkln, [TS, M]),
        ("bu_b", has_bu, [TS, D]),
    ):
        if used:
            opt[name] = nc.declare_dram_parameter(name, shape, f32,
                                                  isOutput=False)
    y = nc.declare_dram_parameter("y", [HALF, D], f32, isOutput=True)

    with tile.TileContext(nc) as tc, ExitStack() as ctx:
        wpool = ctx.enter_context(tc.tile_pool(name="weights", bufs=1))

        wp_sb = wpool.tile([TS, WPACK_COLS], fp8)
        nc.sync.dma_start(wp_sb[:], wpack[:])
        _off = [0]

        def _wseg(nk, ncols):
            a = _off[0]
            _off[0] += nk * ncols
            return wp_sb[:, a:_off[0]].rearrange("p (k m) -> p k m", k=nk)

        wd_sb = _wseg(8, M)     # [128, 8, 512]: d-blk k -> Wd[d-blk, :]
        wq_sb = _wseg(4, M)
        wk_sb = _wseg(4, M)
        wv_sb = _wseg(4, M)
        w1_sb = _wseg(4, M)
        w2_sb = _wseg(4, M)
        wu_sb = _wseg(4, D)
        tt_sb = _wseg(2, TS)
        id_sb = wpool.tile([TS, TS], bf)
        nc.sync.dma_start(id_sb[:], ident[:])
        hm_sb = wpool.tile([TS, 1], f32)
        nc.sync.dma_start(hm_sb[:], hmask[:])
        opt_sb = {}
        for name, h in opt.items():
            t = wpool.tile([TS, h.shape[1]], f32, tag=name, name=name)
            nc.sync.dma_start(t[:], h[:])
            opt_sb[name] = t

        # SBUF activation pools
        p_xf = ctx.enter_context(tc.tile_pool(name="xf", bufs=9))
        p_xT = ctx.enter_context(tc.tile_pool(name="xT", bufs=2))
        p_hT = ctx.enter_context(tc.tile_pool(name="hT", bufs=2))
        p_kT = ctx.enter_context(tc.tile_pool(name="kT", bufs=2))
        p_a1T = ctx.enter_context(tc.tile_pool(name="a1T", bufs=2))
        p_q = ctx.enter_context(tc.tile_pool(name="q", bufs=18))
        p_v = ctx.enter_context(tc.tile_pool(name="v", bufs=18))
        p_k = ctx.enter_context(tc.tile_pool(name="k", bufs=18))
        p_u = ctx.enter_context(tc.tile_pool(name="u", bufs=18))
        p_rt = ctx.enter_context(tc.tile_pool(name="rt", bufs=3))
        p_y = ctx.enter_context(tc.tile_pool(name="y", bufs=4))
        p_sm = ctx.enter_context(tc.tile_pool(name="sm", bufs=8))
        p_z = ctx.enter_context(tc.tile_pool(name="z", bufs=6))
        # PSUM: pt(2 bf16 banks) + mm(4) + out(2) = 8 banks
        p_pt = ctx.enter_context(tc.tile_pool(name="pt", bufs=2,
                                              space="PSUM"))
        p_mm = ctx.enter_context(tc.tile_pool(name="mm", bufs=4,
                                              space="PSUM"))
        p_out = ctx.enter_context(
            tc.tile_pool(name="out", bufs=2, space="PSUM"))

        def rsqrt_dve(xv, nlan, tag):
            """[128, nlan] f32 = rsqrt(xv) on DVE only (xv consumed)."""
            # seed y0 = bits(MAGIC - bits(x)/2); integer ALU on DVE is not
            # available, so do the bit arithmetic in float value domain
            # (|error| < 128 ulp of bit-space -- Newton absorbs it).
            yv = p_sm.tile([TS, nlan], f32, tag=f"{tag}_y")
            t1 = p_sm.tile([TS, nlan], f32, tag=f"{tag}_t")
            t2 = p_sm.tile([TS, nlan], f32, tag=f"{tag}_t2")
            nc.vector.tensor_copy(t1[:], xv[:].bitcast(u32))
            nc.vector.tensor_scalar(t2[:], t1[:], -0.5, float(MAGIC),
                                    ALU.mult, ALU.add)
            nc.vector.tensor_copy(yv[:].bitcast(u32), t2[:])
            for _ in range(1):  # y <- y*(1.5 - 0.5*x*y^2)
                nc.vector.tensor_mul(t1[:], yv[:], yv[:])
                nc.vector.scalar_tensor_tensor(t1[:], t1[:], -0.5, xv[:],
                                               ALU.mult, ALU.mult)
                nc.vector.tensor_scalar_add(t1[:], t1[:], 1.5)
                nc.vector.tensor_mul(yv[:], yv[:], t1[:])
            return yv

        def ln_pair(z_list, gb_bb):
            """LN over free dim for [zq, zk] (or [zk]) PSUM tiles with one
            shared Newton. The PSUM banks are released after only the
            stats read + a bf16 stash copy (~1us), NOT the full LN chain,
            so the next tile's matmuls get their bank immediately.
            k applies on ACT first (kT needs it); q on DVE."""
            n = len(z_list)
            pools = [p_q, p_k][-n:]
            tags = ["q", "k"][-n:]
            ags, zbs = [], []
            for i, z_ps in enumerate(z_list):
                st = p_sm.tile([TS, 6], f32, tag="bnst")
                nc.vector.bn_stats(st[:], z_ps[:])
                zb = p_z.tile([TS, M], bf, tag=f"z{tags[i]}")
                nc.scalar.copy(zb[:], z_ps[:])
                zbs.append(zb)
                ag = p_sm.tile([TS, 2], f32, tag="bnag")
                nc.vector.bn_aggr(ag[:], st[:])
                ags.append(ag)
            xv = p_sm.tile([TS, n], f32, tag="lnx")
            for i, ag in enumerate(ags):
                nc.vector.tensor_scalar_add(xv[:, i:i + 1], ag[:, 1:2],
                                            float(LN_EPS))
            rs = rsqrt_dve(xv, n, "ln")
            outs = []
            for i in reversed(range(n)):
                zb, ag = zbs[i], ags[i]
                gb, bb = gb_bb[i]
                nmr = p_sm.tile([TS, 1], f32, tag=f"nmr{i}")
                nc.vector.scalar_tensor_tensor(nmr[:], ag[:, 0:1], -1.0,
                                               rs[:, i:i + 1],
                                               ALU.mult, ALU.mult)
                o = pools[i].tile([TS, M], bf, tag=tags[i], name=tags[i])
                if i == n - 1:
                    nc.scalar.activation(o[:], zb[:], AF.Identity,
                                         bias=nmr[:], scale=rs[:, i:i + 1])
                else:
                    nc.vector.tensor_scalar(o[:], zb[:], rs[:, i:i + 1],
                                            nmr[:], ALU.mult, ALU.add)
                if gb is not None:
                    nc.vector.tensor_mul(o[:], o[:], gb[:])
                if bb is not None:
                    nc.vector.tensor_add(o[:], o[:], bb[:])
                outs.insert(0, o)
            return outs

        # state carried between phases
        stA = {}           # per-group dict from phase A
        u_prev = [None]

        WMAX = GW * TS

        def phase_a(g0, gn):
            W = gn * TS
            # xT straight from DRAM (host already transposed + fp8-cast)
            xT = p_xT.tile([TS, 8 * WMAX], fp8, tag="xT")
            s0 = g0 * TS
            src = x_t8[:, :].rearrange("p (k s) -> p k s", k=8)[
                :, :, s0:s0 + W]
            dst = xT.rearrange("p (k w) -> p k w", k=8)[:, :, 0:W]
            nc.sync.dma_start(dst, src)

            # hT[m, s] += Wd[d,m].T @ xT[d, s]  (DoubleRow: K=256/mm)
            # s split at 512 (DR moving operand is 2x the out width)
            xTr = xT.rearrange("p (k w) -> p k w", k=8)
            hT = p_hT.tile([TS, 4 * WMAX], fp8, tag="hT")
            for mb in range(4):
                for sh in range(0, W, 512):
                    Wc = min(512, W - sh)
                    acc = p_mm.tile([TS, 4 * TS], f32, tag="mm",
                                    name="hT_ps")
                    for g in range(4):
                        nc.tensor.matmul(
                            acc[:, 0:Wc],
                            wd_sb[:, 2 * g:2 * g + 2, mb * TS:(mb + 1) * TS],
                            xTr[:, 2 * g:2 * g + 2, sh:sh + Wc],
                            start=(g == 0), stop=(g == 3), perf_mode=DR)
                    if has_bd:
                        nc.vector.tensor_scalar_add(
                            acc[:, 0:Wc], acc[:, 0:Wc],
                            opt_sb["bd_c"][:, mb:mb + 1])
                    nc.scalar.copy(
                        hT[:, mb * WMAX + sh:mb * WMAX + sh + Wc],
                        acc[:, 0:Wc])

            qs, vs, ks = [], [], []
            for j in range(gn):
                t_idx = g0 + j
                halo = (t_idx == 0)

                hTr = hT.rearrange("p (k w) -> p k w", k=4)

                def qkv_mm(w3d, name):
                    zp = p_mm.tile([TS, M], f32, tag="mm", name=name)
                    for g in range(2):
                        nc.tensor.matmul(
                            zp[:, 0:M],
                            hTr[:, 2 * g:2 * g + 2, j * TS:(j + 1) * TS],
                            w3d[:, 2 * g:2 * g + 2, 0:M],
                            start=(g == 0), stop=(g == 1), perf_mode=DR)
                    return zp

                zk = qkv_mm(wk_sb, "zk")
                if has_bk:
                    nc.vector.tensor_add(zk[:], zk[:], opt_sb["bk_b"][:])
                if not halo:
                    zq = qkv_mm(wq_sb, "zq")
                    if has_bq:
                        nc.vector.tensor_add(zq[:], zq[:], opt_sb["bq_b"][:])
                    q_sb, k_sb = ln_pair(
                        [zq, zk],
                        [(opt_sb.get("gq_b"), opt_sb.get("bqln_b")),
                         (opt_sb.get("gk_b"), opt_sb.get("bkln_b"))])
                    qs.append(q_sb)
                else:
                    qs.append(None)
                    (k_sb,) = ln_pair(
                        [zk], [(opt_sb.get("gk_b"), opt_sb.get("bkln_b"))])
                ks.append(k_sb)
                zv = qkv_mm(wv_sb, "zv")
                v_sb = p_v.tile([TS, M], bf, tag="v")
                nc.scalar.copy(v_sb[:], zv[:])
                if has_bv:
                    nc.vector.tensor_add(v_sb[:], v_sb[:], opt_sb["bv_b"][:])
                vs.append(v_sb)

            return dict(g0=g0, gn=gn, W=W, hT=hT, qs=qs, vs=vs, ks=ks)

        def phase_a2(st):
            """kT transposes for the whole group (emitted one a-phase
            later, so the LN chain latency is long since hidden)."""
            gn = st["gn"]
            kT = p_kT.tile([TS, 4 * WMAX], fp8, tag="kT")
            for j in range(gn):
                ps = p_pt.tile([TS, 4 * TS], bf, tag="pt", name="ps_k")
                k_sb = st["ks"][j]
                for mb in range(4):
                    nc.tensor.transpose(ps[:, mb * TS:(mb + 1) * TS],
                                        k_sb[:, mb * TS:(mb + 1) * TS],
                                        id_sb[:])
                dst = kT.rearrange("p (k w) -> p k w", k=4)[
                    :, :, j * TS:(j + 1) * TS]
                src = ps[:].rearrange("p (k w) -> p k w", k=4)
                nc.scalar.copy(dst, src)
            st["kT"] = kT

        def phase_b(st):
            g0, gn, W = st["g0"], st["gn"], st["W"]
            xfs = []
            for j in range(gn):
                t_idx = g0 + j
                if t_idx > 0:
                    xf = p_xf.tile([TS, D], f32, tag="xf")
                    nc.sync.dma_start(
                        xf[:], x_f32[(t_idx - 1) * TS:t_idx * TS, :])
                    xfs.append(xf)
                else:
                    xfs.append(None)
            kTr = st["kT"].rearrange("p (k w) -> p k w", k=4)
            # a1T[m1, s] = gelu(W1[m,m1].T @ kT[m, s])
            a1T = p_a1T.tile([TS, 4 * WMAX], fp8, tag="a1T")
            for m1b in range(4):
                for sh in range(0, W, 512):
                    Wc = min(512, W - sh)
                    acc = p_mm.tile([TS, 4 * TS], f32, tag="mm",
                                    name="a1_ps")
                    for g in range(2):
                        nc.tensor.matmul(
                            acc[:, 0:Wc],
                            w1_sb[:, 2 * g:2 * g + 2,
                                  m1b * TS:(m1b + 1) * TS],
                            kTr[:, 2 * g:2 * g + 2, sh:sh + Wc],
                            start=(g == 0), stop=(g == 1), perf_mode=DR)
                    nc.scalar.activation(
                        a1T[:, m1b * WMAX + sh:m1b * WMAX + sh + Wc],
                        acc[:, 0:Wc], AF.Gelu_apprx_tanh)

            # breadth-first over the group's tiles so each PE stage's DVE
            # dependencies were produced a stage earlier
            a1Tr = a1T.rearrange("p (k w) -> p k w", k=4)
            preds, us, rtrs, rTs = [], [], [], []
            for j in range(gn):
                pred = p_mm.tile([TS, M], f32, tag="mm", name="pred")
                for g in range(2):
                    nc.tensor.matmul(
                        pred[:, 0:M],
                        a1Tr[:, 2 * g:2 * g + 2, j * TS:(j + 1) * TS],
                        w2_sb[:, 2 * g:2 * g + 2, 0:M],
                        start=(g == 0), stop=(g == 1), perf_mode=DR)
                preds.append(pred)
                u_sb = p_u.tile([TS, M], fp8, tag="u")
                nc.vector.tensor_sub(u_sb[:], st["vs"][j][:], pred[:])
                if g0 + j == 0:
                    nc.vector.tensor_scalar_mul(u_sb[:], u_sb[:],
                                                hm_sb[:, 0:1])
                us.append(u_sb)
            for j in range(gn):
                if g0 + j == 0:
                    u_prev[0] = us[j]
                    rtrs.append(None)
                    continue
                mem = p_mm.tile([TS, M], f32, tag="mm", name="mem")
                nc.tensor.matmul(mem[:], tt_sb[:, 0, :], u_prev[0][:],
                                 start=True, stop=False)
                nc.tensor.matmul(mem[:], tt_sb[:, 1, :], us[j][:],
                                 start=False, stop=True)
                u_prev[0] = us[j]
                rtr = p_rt.tile([TS, M], bf, tag="rtr")
                nc.vector.tensor_mul(rtr[:], st["qs"][j][:], mem[:])
                rtrs.append(rtr)  # 64x-scaled (SCL inside the T matrices)
            for j in range(gn):
                if rtrs[j] is None:
                    rTs.append(None)
                    continue
                ps = p_pt.tile([TS, 4 * TS], bf, tag="pt", name="ps_r")
                for mb in range(4):
                    nc.tensor.transpose(ps[:, mb * TS:(mb + 1) * TS],
                                        rtrs[j][:, mb * TS:(mb + 1) * TS],
                                        id_sb[:])
                rT = p_rt.tile([TS, 4 * TS], fp8, tag="rT")
                nc.scalar.copy(rT[:], ps[:])
                rTs.append(rT)
            for j in range(gn):
                if rTs[j] is None:
                    continue
                t_idx = g0 + j
                rTr = rTs[j].rearrange("p (k w) -> p k w", k=4)
                # out in two 1-bank halves (p_out bufs=2): the next half's
                # matmuls only wait for a half-sized y, halving the
                # PE<->DVE lockstep granularity at the same PSUM budget
                for nb in range(2):
                    cols = slice(nb * 512, (nb + 1) * 512)
                    out_ps = p_out.tile([TS, 512], f32, tag="out")
                    for g in range(2):
                        nc.tensor.matmul(
                            out_ps[:],
                            rTr[:, 2 * g:2 * g + 2, 0:TS],
                            wu_sb[:, 2 * g:2 * g + 2, cols],
                            start=(g == 0), stop=(g == 1), perf_mode=DR)
                    y_sb = p_y.tile([TS, 512], f32, tag="y")
                    # y = x + out/SCL  (out carries the 64x mem scaling)
                    nc.vector.scalar_tensor_tensor(
                        y_sb[:], out_ps[:], 1.0 / SCL, xfs[j][:, cols],
                        ALU.mult, ALU.add)
                    if has_bu:
                        nc.vector.tensor_add(y_sb[:], y_sb[:],
                                             opt_sb["bu_b"][:, cols])
                    nc.sync.dma_start(y[(t_idx - 1) * TS:t_idx * TS, cols],
                                      y_sb[:])

        # software pipeline: A0 K0* A1 B0 K1 A2 B1 ...
        # K(g-1) BEFORE A(g): kT copies land at the ACT queue front (their
        # deps are a full phase old); B(g-1) after A(g) for PE backfill.
        prev = None
        for (g0, gn) in GROUPS:
            if prev is not None:
                phase_a2(prev)
            cur = phase_a(g0, gn)
            if prev is not None:
                phase_b(prev)
            prev = cur
        phase_a2(prev)
        phase_b(prev)

    _fix_matmult_waits(nc)
    return nc


def _prep_inputs(x, Wd, bd, Wq, bq, Wk, bk, Wv, bv, gq, bq_ln, gk, bk_ln,
                 W1, W2, Wu, bu, adaptive_lr, forget_factor):
    """Host-side: flags, decay matrices, per-core slabs, bf16 packing."""
    f = np.float32
    bd, bq, bk, bv, bu = (np.asarray(a, f) for a in (bd, bq, bk, bv, bu))
    gq, bq_ln, gk, bk_ln = (np.asarray(a, f) for a in (gq, bq_ln, gk, bk_ln))
    flags = (bool(bd.any()), bool(bq.any()), bool(bk.any()), bool(bv.any()),
             bool((gq != 1).any()), bool(bq_ln.any()),
             bool((gk != 1).any()), bool(bk_ln.any()), bool(bu.any()))

    g = 1.0 / (1.0 + np.exp(-np.float64(forget_factor)))
    lr = np.float64(adaptive_lr)
    t_idx = np.arange(TS)
    lag_cur = t_idx[:, None] - t_idx[None, :]
    Tcur = np.where(lag_cur >= 0, g ** np.maximum(lag_cur, 0), 0.0) * lr * SCL
    lag_prev = t_idx[:, None] + TS - t_idx[None, :]
    Tprev = (g ** lag_prev) * lr * SCL
    TT = np.concatenate([Tprev, Tcur], axis=1).T.astype(f)  # [256, 128]

    def seg(w):
        w = np.asarray(w, f)          # [K, N] -> [128, nk*N]
        nk = w.shape[0] // TS
        return w.reshape(nk, TS, w.shape[1]).transpose(1, 0, 2).reshape(TS, -1)

    wpack = np.ascontiguousarray(np.concatenate(
        [seg(w) for w in (Wd, Wq, Wk, Wv, W1, W2, Wu, TT)],
        axis=1)).astype(np_fp8)
    common = {
        "wpack": wpack,
        "ident": np.eye(TS, dtype=f).astype(np_bf16),
    }
    names = ("bd_c", "bq_b", "bk_b", "bv_b", "gq_b", "bqln_b", "gk_b",
             "bkln_b", "bu_b")
    vecs = (bd, bq, bk, bv, gq, bq_ln, gk, bk_ln, bu)
    for name, used, vec in zip(names, flags, vecs):
        if not used:
            continue
        if name == "bd_c":
            common[name] = np.ascontiguousarray(
                vec.reshape(4, TS).T, f)      # [128, 4]: col mb = bd block
        else:
            common[name] = np.ascontiguousarray(
                np.broadcast_to(vec, (TS, vec.shape[0])), f)

    x = np.asarray(x, f)
    in_maps = []
    for c in range(N_CORES):
        b, sh = c // 2, c % 2
        if sh == 0:
            haloblk = np.zeros((TS, D), f)
            hm = np.zeros((TS, 1), f)
        else:
            haloblk = x[b, HALF - TS:HALF]
            hm = np.ones((TS, 1), f)
        slab = np.concatenate([haloblk, x[b, sh * HALF:(sh + 1) * HALF]],
                              axis=0)
        m = dict(common)
        # [SLAB, D] -> transpose -> [8, 128, SLAB] -> [128, 8*SLAB] fp8
        xt = np.ascontiguousarray(slab.T).reshape(8, TS, SLAB)
        m["x_t8"] = np.ascontiguousarray(
            xt.transpose(1, 0, 2).reshape(TS, 8 * SLAB)).astype(np_fp8)
        m["x_f32"] = np.ascontiguousarray(x[b, sh * HALF:(sh + 1) * HALF])
        m["hmask"] = hm
        in_maps.append(m)
    return flags, in_maps


def kernel(**inputs):
    global LAST_RESULTS
    flags, in_maps = _prep_inputs(**inputs)
    if flags not in _PROG_CACHE:
        _PROG_CACHE[flags] = _build_program(flags)
    nc = _PROG_CACHE[flags]

    res = run_bass_kernel_spmd(nc, in_maps, list(range(N_CORES)),
                               trace=TRACE, trace_kwargs=TRACE_KWARGS)
    LAST_RESULTS = res

    out = np.empty((B, S, D), np.float32)
    for c in range(N_CORES):
        b, sh = c // 2, c % 2
        out[b, sh * HALF:(sh + 1) * HALF] = res.results[c]["y"]
    return out
